# revision 9
# baseline (speedup 1.0000x reference)
"""Trainium2 Bass kernel for the nn_Dynamics problem (v3).

Math (per batch element, d=8, H=128):
  x = X[:, :8], v = X[:, 8:]
  h0 = tanh(W0 x + b0); h1 = tanh(W1 h0 + b1)
  A0 = W1^T (w2*(1-h1^2))  [via  c0 - (W1*w2) @ h1^2, c0 = W1^T w2]
  a0' = (h0^2-1)*A0 = -a0;  g' = W0^T a0' = -g
  h0p' = (h0^2-1)*t0 (t0 = W0 v); t1c = W1 h0p' = -t1
  e1' = (1-h1^2)*h1*t1c^2 = e1/w2;  w = h0*t0^2;  e2c = a0'*w = -A0*u
  hvv = sum_k -2*w2[k]*e1'[k] + 2*e2c[k]
  p = -(K x + D v)
  out = p + g'*(hvv - g'.p)/(1+|g|^2)   (Sherman-Morrison)

Layout: features on partitions, batch on the free axis. Supertiles of 1024
batch columns (2 matmul tiles of 512). X is transposed host-side and shipped
as ONE [16, BC] strip (x rows 0:8, v rows 8:16), DMA'd in per-supertile
chunks so compute starts as soon as chunk 0 lands. z0/t0/force all stream
the same XTF chunk with stacked [16, ...] stationary weights. Weights are
consolidated into two DMAs (Wsm [16,264] + Wbig [128,352]) issued from the
Scalar sequencer so they overlap the sync-issued X chunks.

front(s): z0/t0/force MMs, per-half tanh0, h0p, w, z1, tanh1, e1, h1sq,
A0n, a0, e2.  tail(s) (per 512-half): g + hv MMs into fm, E copy (72 rows),
4 PE transposes, tl copy, batch-major tail math on gpsimd/vector, out DMA.
tail(s) is emitted after front(s+1) so chain-head ops of s+1 get engine-queue
priority over the long tail of s.

Sharding: pure data parallel over 8 NeuronCores (8192 rows each), weights
replicated, outputs concatenated.
"""

import os

import numpy as np

import concourse.bacc as bacc
import concourse.bass as bass
import concourse.dve_ops as dve_ops
import concourse.tile as tile
from concourse import mybir
from concourse.bass_utils import run_bass_kernel_spmd
from concourse.dve_ops import DveOp
from concourse.dve_ops import has_src1
from concourse.dve_spec import C0, One, Spec, Src0, Src1, lower, sq
from concourse.masks import make_identity

F32 = mybir.dt.float32
F16 = mybir.dt.float16
AX = mybir.AxisListType
OP = mybir.AluOpType
ACT = mybir.ActivationFunctionType

DIM = 8
H = 128
BATCH = 65536
NCORES = 8
BC = BATCH // NCORES          # 8192 rows per core
ST = 1024                     # supertile width (batch cols)
NST = BC // ST                # 8 supertiles
TW = 512                      # matmul tile width (one PSUM bank of f32)
NCH = ST // 128               # 8 transpose chunks per supertile
HCH = NCH // 2                # 4 transpose chunks per half
ER = 72                       # E rows carried through the transpose

LAST_RESULTS = None

# ---------------- custom fused DVE ops ----------------


def _register_op(name, body, reference):
    if name in dve_ops._SUB_OPCODE_FOR_NAME:
        for op in dve_ops.OPS:
            if op.name == name:
                return op
    from concourse.dve_uop import DveOpSpec

    spec = Spec(body=body, reference=reference)
    shas = {}
    for ver in ("v3", "v4"):
        shas[ver] = DveOpSpec(
            name=name,
            opcode=dve_ops._CUSTOM_DVE_ROW_BASE + len(dve_ops.OPS),
            uops=lower(spec, ver=ver),
            rd1_en=has_src1(spec),
        ).sha(ver)
    op = DveOp(name, spec, subdim=False, uops_sha=shas)
    dve_ops.OPS.append(op)
    dve_ops.CUSTOM_DVE_SPECS[name] = spec
    dve_ops._SUB_OPCODE_FOR_NAME[name] = (
        dve_ops._CUSTOM_DVE_ROW_BASE + len(dve_ops.OPS) - 1
    )
    return op


# h0p' = (h0^2 - 1) * t0     (also a0' = (h0^2 - 1) * A0s)
OP_SQM1_MUL = _register_op(
    "ANT_SQM1_MUL",
    (sq(Src0) - One) * Src1,
    lambda in0, in1: (in0 * in0 - 1.0) * in1,
)
# w = h0 * t0^2
OP_WOP = _register_op(
    "ANT_WOP",
    Src0 * sq(Src1),
    lambda in0, in1: in0 * in1 * in1,
)
# a0' = (h0^2 - 1) * (A0n + c0)
OP_A0F = _register_op(
    "ANT_A0F",
    (sq(Src0) - One) * (Src1 + C0),
    lambda in0, in1, s0: (in0 * in0 - 1.0) * (in1 + s0),
)
# e1' = (1 - h1^2) * h1 * t1^2
OP_E1G = _register_op(
    "ANT_E1G",
    (One - sq(Src0)) * Src0 * sq(Src1),
    lambda in0, in1: (1.0 - in0 * in0) * in0 * in1 * in1,
)


def build_nc():
    nc = bacc.Bacc()

    XTd = nc.dram_tensor("XTd", [16, BC], F16, kind="ExternalInput")
    Wsm = nc.dram_tensor("Wsm", [16, 264], F16, kind="ExternalInput")
    Wbig = nc.dram_tensor("Wbig", [H, 352], F16, kind="ExternalInput")
    cb = nc.dram_tensor("cb", [H, 3], F32, kind="ExternalInput")
    out = nc.dram_tensor("out", [BC, DIM], F32, kind="ExternalOutput")

    from contextlib import ExitStack

    with tile.TileContext(nc) as tc, ExitStack() as stk:
        consts = stk.enter_context(tc.tile_pool(name="consts", bufs=1))
        work = stk.enter_context(tc.tile_pool(name="work", bufs=2))
        ps = stk.enter_context(tc.tile_pool(name="ps", bufs=1, space="PSUM"))

        # ---------------- inputs: chunked X + consolidated weights --------
        # X chunk 0 first on the sync queue so the first z0 can start ASAP;
        # the weight DMAs issue concurrently from the Scalar sequencer.
        # Supertile 0 arrives as two half-tiles so the very first z0 matmul
        # only waits on a 16KB transfer; later supertiles use full chunks.
        XTFH = {}
        for s in range(NST):
            if s == 0:
                XTFH[s] = [
                    consts.tile([16, TW], F16, name=f"XTFa_{s}"),
                    consts.tile([16, TW], F16, name=f"XTFb_{s}"),
                ]
            else:
                XTFH[s] = consts.tile([16, ST], F16, name=f"XTF_{s}")
        nc.sync.dma_start(out=XTFH[0][0], in_=XTd[:, 0:TW])
        nc.sync.dma_start(out=XTFH[0][1], in_=XTd[:, TW:ST])

        Wsm_sb = consts.tile([16, 264], F16)
        nc.scalar.dma_start(out=Wsm_sb, in_=Wsm[:, :])
        Wbig_sb = consts.tile([H, 352], F16)
        nc.scalar.dma_start(out=Wbig_sb, in_=Wbig[:, :])
        cb_sb = consts.tile([H, 3], F32)
        nc.scalar.dma_start(out=cb_sb, in_=cb[:, :])

        for s in range(1, NST):
            nc.sync.dma_start(out=XTFH[s], in_=XTd[:, s * ST : (s + 1) * ST])

        def xhalf(s, h):
            if s == 0:
                return XTFH[0][h]
            return XTFH[s][:, h * TW : (h + 1) * TW]

        Wz = Wsm_sb[:, 0:H]
        Wt = Wsm_sb[:, H : 2 * H]
        WK = Wsm_sb[:, 2 * H : 2 * H + DIM]
        W1T_sb = Wbig_sb[:, 0:H]
        Wa_sb = Wbig_sb[:, H : 2 * H]
        W0p_sb = Wbig_sb[:, 2 * H : 2 * H + 32]
        hvW_sb = Wbig_sb[:, 2 * H + 32 : 2 * H + 96]

        b0c = cb_sb[:, 0:1]
        b1c = cb_sb[:, 1:2]
        c0c = cb_sb[:, 2:3]

        identF = consts.tile([128, 128], F32)
        make_identity(nc, identF)
        identH = consts.tile([128, 128], F16)
        nc.scalar.copy(identH, identF)

        out_sb = consts.tile([128, NST * NCH * DIM], F32)

        # tail scratch: Q[p, chunk, grp, 10]; grp0 = [g^2 x8, 1, 0] -> den,
        # grp1 = [-g*p x8, hvv, 0] -> num. Cols 8/9 of grp0 and col 9 of
        # grp1 are preset once (1.0 / 0.0) and never overwritten.
        Q = consts.tile([128, NCH * 20], F32)
        nc.vector.memset(Q, 0.0)
        Qv = Q.rearrange("p (c g k) -> p c g k", g=2, k=10)
        nc.vector.memset(Qv[:, :, 0, 8:9], 1.0)

        # ---------------- main loop (software-pipelined emission) ---------
        state = {}

        fa = {}

        def front_a(s):
            # z0/t0/force: same moving chunk, stacked [16, .] weights.
            # Same-weight pairs adjacent so walrus can skip LD_WEIGHTS.
            zh = []
            for h in range(2):
                z = ps.tile([128, TW], F32, tag="z", bufs=2, name=f"z_{s}_{h}")
                nc.tensor.matmul(z, Wz, xhalf(s, h), start=True, stop=True)
                zh.append(z)
            t = ps.tile([128, ST], F32, tag="t", name=f"t_{s}")
            for h in range(2):
                nc.tensor.matmul(
                    t[:, h * TW : (h + 1) * TW], Wt, xhalf(s, h),
                    start=True, stop=True,
                )
            fmh = []
            for h in range(2):
                fm = ps.tile([128, TW], F32, tag="fm", bufs=2, name=f"fm_{s}_{h}")
                nc.tensor.matmul(
                    fm[0:8, :], WK, xhalf(s, h), start=True, stop=True,
                )
                fmh.append(fm)
            fa[s] = (zh, t, fmh)

        def front_b(s):
            zh, t, fmh = fa.pop(s)

            h0 = work.tile([128, ST], F16, tag="h0", name=f"h0_{s}")
            for h in range(2):
                lo = slice(h * TW, (h + 1) * TW)
                nc.scalar.activation(h0[:, lo], zh[h], ACT.Tanh, bias=b0c, scale=1.0)
            h0p = work.tile([128, ST], F16, tag="h0p", name=f"h0p_{s}")
            nc.vector._custom_dve(OP_SQM1_MUL, out=h0p, in0=h0, in1=t[:, :])
            w = work.tile([128, ST], F16, tag="w", name=f"w_{s}")
            nc.vector._custom_dve(OP_WOP, out=w, in0=h0, in1=t[:, :])

            z1h = []
            for h in range(2):
                lo = slice(h * TW, (h + 1) * TW)
                z1 = ps.tile([128, TW], F32, tag="z", bufs=2, name=f"z1_{s}_{h}")
                nc.tensor.matmul(z1, W1T_sb, h0[:, lo], start=True, stop=True)
                z1h.append(z1)
            t1h = []
            for h in range(2):
                lo = slice(h * TW, (h + 1) * TW)
                t1 = ps.tile([128, TW], F32, tag="a", bufs=2, name=f"t1_{s}_{h}")
                nc.tensor.matmul(t1, W1T_sb, h0p[:, lo], start=True, stop=True)
                t1h.append(t1)
            h1 = work.tile([128, ST], F16, tag="h1", name=f"h1_{s}")
            for h in range(2):
                lo = slice(h * TW, (h + 1) * TW)
                nc.scalar.activation(h1[:, lo], z1h[h], ACT.Tanh, bias=b1c, scale=1.0)
            # e1 per half, before h1sq/a0, so the A0n allocs (same PSUM tag)
            # don't wait behind a full-width e1, and h1sq (V) overlaps S's tanh
            e1 = work.tile([128, ST], F16, tag="e1", name=f"e1_{s}")
            for h in range(2):
                lo = slice(h * TW, (h + 1) * TW)
                nc.vector._custom_dve(
                    OP_E1G, out=e1[:, lo], in0=h1[:, lo], in1=t1h[h]
                )
            h1sq = work.tile([128, ST], F16, tag="h1sq", name=f"h1sq_{s}")
            for h in range(2):
                lo = slice(h * TW, (h + 1) * TW)
                nc.vector.tensor_mul(h1sq[:, lo], h1[:, lo], h1[:, lo])

            a0 = work.tile([128, ST], F16, tag="a0", name=f"a0_{s}")
            for h in range(2):
                lo = slice(h * TW, (h + 1) * TW)
                A0n = ps.tile([128, TW], F32, tag="a", bufs=2, name=f"A0n_{s}_{h}")
                nc.tensor.matmul(A0n, Wa_sb, h1sq[:, lo], start=True, stop=True)
                nc.vector._custom_dve(
                    OP_A0F, out=a0[:, lo], in0=h0[:, lo], in1=A0n, s0=c0c
                )
            e2 = work.tile([128, ST], F16, tag="e2", name=f"e2_{s}")
            for h in range(2):
                lo = slice(h * TW, (h + 1) * TW)
                nc.vector.tensor_mul(e2[:, lo], a0[:, lo], w[:, lo])
            state[s] = (fmh, e1, a0, e2)

        def tail_mm(s):
            fmh, e1, a0, e2 = state[s]
            # g for both halves first (shared W0p weights), then the hv
            # accumulation pairs (e1->e2 chained into the same bank).
            # Emitted right after front(s+1)'s z0/t0/force block so the
            # transposes (end of tail_rest) have plenty of PE work queued
            # ahead of them while the E copies run on Scalar.
            for h in range(2):
                lo = slice(h * TW, (h + 1) * TW)
                nc.tensor.matmul(
                    fmh[h][32:64, :], W0p_sb, a0[:, lo],
                    start=True, stop=True, tile_position=(0, 32),
                )
            for h in range(2):
                lo = slice(h * TW, (h + 1) * TW)
                nc.tensor.matmul(
                    fmh[h][64:96, :], hvW_sb[:, 0:32], e1[:, lo],
                    start=True, stop=False, tile_position=(0, 64),
                )
                nc.tensor.matmul(
                    fmh[h][64:96, :], hvW_sb[:, 32:64], e2[:, lo],
                    start=False, stop=True, tile_position=(0, 64),
                )

        def tail_rest(s):
            fmh, e1, a0, e2 = state.pop(s)
            E = work.tile([128, ST], F16, tag="E", name=f"E_{s}")
            bm = ps.tile([128, NCH * ER], F16, tag="a", bufs=2, name=f"bm_{s}")
            for h in range(2):
                lo = slice(h * TW, (h + 1) * TW)
                nc.scalar.copy(E[0:ER, lo], fmh[h][0:ER, :])
                for c in range(HCH * h, HCH * (h + 1)):
                    nc.tensor.transpose(
                        bm[:, ER * c : ER * (c + 1)],
                        E[0:ER, 128 * c : 128 * (c + 1)],
                        identH[0:ER, 0:ER],
                    )

            tl = work.tile([128, NCH * 24], F32, tag="tl", name=f"tl_{s}")
            tl4 = tl.rearrange("p (c q f) -> p c q f", q=3, f=DIM)
            for h in range(2):
                ch = slice(HCH * h, HCH * (h + 1))
                bmsrc = bass.AP(
                    tensor=bm.tensor,
                    offset=bm.offset + ER * HCH * h,
                    ap=[list(bm.ap[0]), [ER, HCH], [32, 3], [1, DIM]],
                )
                nc.scalar.copy(tl4[:, ch], bmsrc)

                p3 = tl4[:, ch, 0, :]
                g3 = tl4[:, ch, 1, :]
                hv1 = tl4[:, ch, 2, 0:1]

                # strips hold -p, +g, -hvv (host-side sign flips), so the
                # tail is all plain ops:
                # num' = -g.p - hvv = -num; su = g*(num'*rec) = -g*s
                nc.gpsimd.tensor_mul(Qv[:, ch, 0, 0:8], g3, g3)
                nc.gpsimd.tensor_mul(Qv[:, ch, 1, 0:8], g3, p3)
                nc.gpsimd.tensor_copy(Qv[:, ch, 1, 8:9], hv1)
                R = work.tile([128, HCH * 2], F32, tag="R", bufs=4,
                              name=f"R_{s}_{h}")
                Rv = R.rearrange("p (c g) -> p c g", g=2)
                nc.vector.tensor_reduce(Rv, Qv[:, ch], axis=AX.X, op=OP.add)

                rec = work.tile([128, HCH], F32, tag="rec", bufs=4,
                                name=f"rec_{s}_{h}")
                nc.vector.reciprocal(
                    rec, Rv[:, :, 0:1].rearrange("p c g -> p (c g)")
                )
                s4 = work.tile([128, HCH], F32, tag="s4", bufs=4,
                               name=f"s4_{s}_{h}")
                nc.gpsimd.tensor_mul(
                    s4, Rv[:, :, 1:2].rearrange("p c g -> p (c g)"), rec
                )
                s4b = bass.AP(
                    tensor=s4.tensor,
                    offset=s4.offset,
                    ap=[list(s4.ap[0]), [1, HCH], [0, DIM]],
                )
                su = work.tile([128, HCH * DIM], F32, tag="su", bufs=4,
                               name=f"su_{s}_{h}")
                su3 = su.rearrange("p (c f) -> p c f", f=DIM)
                nc.gpsimd.tensor_mul(su3, g3, s4b)
                ob = out_sb[
                    :, NCH * DIM * s + HCH * DIM * h : NCH * DIM * s
                    + HCH * DIM * (h + 1)
                ]
                ob3 = ob.rearrange("p (c f) -> p c f", f=DIM)
                nc.gpsimd.tensor_sub(ob3, su3, p3)  # out = su - (-p)

                oap = out[:, :]
                dst = bass.AP(
                    tensor=oap.tensor,
                    offset=oap.offset + (s * ST + h * TW) * DIM,
                    ap=[[DIM, 128], [128 * DIM, HCH], [1, DIM]],
                )
                nc.sync.dma_start(out=dst, in_=ob3)

        for s in range(NST):
            front_a(s)
            if s > 0:
                tail_mm(s - 1)
            front_b(s)
            if s > 0:
                tail_rest(s - 1)
        tail_mm(NST - 1)
        tail_rest(NST - 1)

    if not nc.is_finalized():
        nc.finalize()

    return nc


_NC_CACHE = None


def _install_ntff_shim():
    """Register the axon NTFF profile hook (missing antenv.axon_hooks shim)."""
    import sys
    import types

    if "antenv.axon_hooks" in sys.modules:
        return
    try:
        sys.path.insert(0, "/root/.axon_site")
        from trn_agent_boot.trn_boot import _ntff_profile_via_ctypes

        hook = _ntff_profile_via_ctypes("/opt/axon/libaxon_pjrt.so")
        mod = types.ModuleType("antenv.axon_hooks")
        mod.get_axon_ntff_profile_hook = lambda: hook
        sys.modules["antenv.axon_hooks"] = mod
    except Exception:
        pass


def kernel(**inputs):
    global LAST_RESULTS, _NC_CACHE
    trace = bool(int(os.environ.get("KERNEL_TRACE", "0")))
    if trace:
        _install_ntff_shim()
    if _NC_CACHE is None:
        _NC_CACHE = build_nc()
    nc = _NC_CACHE

    X = np.ascontiguousarray(inputs["X"], dtype=np.float32)
    K = np.asarray(inputs["K"], np.float32)
    D = np.asarray(inputs["D"], np.float32)
    W0 = np.asarray(inputs["W0"], np.float32)
    W1 = np.asarray(inputs["W1"], np.float32)
    W2 = np.asarray(inputs["W2"], np.float32)
    w2 = W2.reshape(H)

    wsm = np.zeros((16, 264), np.float32)
    wsm[0:8, 0:H] = W0.T
    wsm[8:16, H : 2 * H] = W0.T
    wsm[0:16, 2 * H : 2 * H + DIM] = np.concatenate([K.T, D.T], axis=0)  # -p

    wbig = np.zeros((H, 352), np.float32)
    wbig[:, 0:H] = W1.T
    wbig[:, H : 2 * H] = -(W1 * w2[:, None])
    wbig[:, 2 * H : 2 * H + 8] = -W0          # g-strip = +g (a0' is -a0)
    wbig[:, 2 * H + 32 : 2 * H + 40] = (2.0 * w2)[:, None]   # hv-strip = -hvv
    wbig[:, 2 * H + 64 : 2 * H + 72] = -2.0

    cbm = np.zeros((H, 3), np.float32)
    cbm[:, 0] = np.asarray(inputs["b0"], np.float32)
    cbm[:, 1] = np.asarray(inputs["b1"], np.float32)
    cbm[:, 2] = W1.T @ w2

    shared = {
        "Wsm": wsm.astype(np.float16),
        "Wbig": wbig.astype(np.float16),
        "cb": cbm,
    }
    in_maps = []
    for i in range(NCORES):
        xc = X[i * BC : (i + 1) * BC]
        xt = np.ascontiguousarray(xc.T).astype(np.float16)  # [16, BC]
        m = {"XTd": xt}
        m.update(shared)
        in_maps.append(m)

    res = run_bass_kernel_spmd(
        nc, in_maps, core_ids=list(range(NCORES)), trace=trace
    )
    LAST_RESULTS = res
    out_full = np.concatenate([res.results[i]["out"] for i in range(NCORES)], axis=0)
    return out_full.astype(np.float32)
